# revision 30
# baseline (speedup 1.0000x reference)
"""Trainium2 Bass kernel for AttentionAggregate (GAT-style neighbor aggregation).

Reference computation (per node n, neighbors k=0..K-1):
    pt = target @ W.T + b                      # [N, D]
    pm = middle @ W.T + b                      # [N, K, D]
    score = leaky_relu((pt[:,None,:] + pm) @ a_w.T + a_b)
    coef  = softmax(score, axis=K)
    out   = sum_k coef * middle                # [N, D]

Key algebraic simplification: the W-projection only enters through the dot
with a_w, so with u = a_w @ W (a single D-vector) and c = 2*(a_w.b) + a_b:
    score[n,k] = target[n].u + middle[n,k].u + c
This removes all large matmuls; the kernel is a memory-bound pass over
`middle` (512 MiB) with per-node softmax weighting.

Sharding: data-parallel over nodes. N=16384 nodes split across 8 cores
(2048 nodes each); W/b/a_w/a_b replicated; no cross-core communication.

Per 128-node tile [128, K, D] (node on partition):
  load: `middle` is cast fp32->fp16 IN THE DMA (gpsimd software DGE can
      cast) -- halves SBUF traffic and enables the DVE 2x mode; HBM reads
      are unchanged (still the roofline, ~12 us/tile over 16 DMA engines).
  scores (DVE): m2 = mh * u (u broadcast via a stride-0 AP; fp16 runs at
      DVE 2x), a 3-level tree of fp16 tensor_adds (tensor_tensor runs 2x;
      tensor_reduce has no fast mode), then a [P,K,32]->[P,K] reduce with
      fp32 output (fp32 accumulation keeps score err ~1e-3).
  softmax: leaky_relu(s + target.u + c) in one ACT Lrelu (bias = per-node
      target score, alpha = slope); e = exp(s2) in fp16 with den = sum_k e
      from the same instruction's f32 accumulator; 1/den on DVE. No
      max-subtraction: scores are O(+-8) so exp is safe (also in fp16).
  aggregation (PE, fp16): out = sum_k diag(e[:,k]) @ mh[:,k,:], 32
      accumulating matmuls (1 cycle/row). The diag stack is built half on
      DVE (ONE broadcast-AP tensor_mul for k < KD: dgs[p,k,q] =
      e[p,k]*id[p,q]) and half on ACT (per-k Copy with per-partition
      scale) to balance engine load. PSUM evacuated with the 1/den
      normalization folded in (ACT Copy, scale=rcp), software-pipelined
      one tile late so no engine round-trip blocks the next tile.
"""

from contextlib import ExitStack

import numpy as np

import concourse.bass as bass
import concourse.tile as tile
from concourse import mybir
from concourse.bass_utils import run_bass_kernel_spmd

N_CORES = 8
N, K, D = 16384, 32, 256
NS = N // N_CORES  # nodes per core
P = 128
F32 = mybir.dt.float32
F16 = mybir.dt.float16
ALU = mybir.AluOpType
AF = mybir.ActivationFunctionType
AX = mybir.AxisListType
NEG_SLOPE = 0.01


def emit_kernel(tc, out, tgt, mid, W, b, a_w, a_b, ident, ns):
    nc = tc.nc
    nt = ns // P  # node tiles per core
    with ExitStack() as ctx:
        singles = ctx.enter_context(tc.tile_pool(name="singles", bufs=1))
        mids = ctx.enter_context(tc.tile_pool(name="mids", bufs=5))
        tree = ctx.enter_context(tc.tile_pool(name="tree", bufs=1))
        small = ctx.enter_context(tc.tile_pool(name="small", bufs=3))
        dgss = ctx.enter_context(tc.tile_pool(name="dgss", bufs=2))
        psum = ctx.enter_context(tc.tile_pool(name="psum", bufs=3, space="PSUM"))
        psums = ctx.enter_context(tc.tile_pool(name="psums", bufs=1, space="PSUM"))
        outs = ctx.enter_context(tc.tile_pool(name="outs", bufs=3))

        # ---- setup: u = a_w @ W, c = 2*(a_w.b) + a_b ----
        W0 = singles.tile([P, D], F32)
        W1 = singles.tile([P, D], F32)
        nc.sync.dma_start(W0, W[0:P, :])
        nc.sync.dma_start(W1, W[P : 2 * P, :])
        # a_w transposed onto partitions: awT[p, g] = a_w[0, g*128 + p]
        awT = singles.tile([P, 2], F32)
        nc.sync.dma_start(awT, a_w.rearrange("o (g p) -> p (g o)", g=2))
        b_row = singles.tile([1, D], F32)
        nc.sync.dma_start(b_row, b.unsqueeze(0))
        aw_row = singles.tile([1, D], F32)
        nc.sync.dma_start(aw_row, a_w)
        ab_t = singles.tile([1, 1], F32)
        nc.sync.dma_start(ab_t, a_b.unsqueeze(0))
        id_t = singles.tile([P, P], F32)
        nc.sync.dma_start(id_t, ident)
        id16 = singles.tile([P, P], F16)
        nc.vector.tensor_copy(id16, id_t)
        # idK2[p, q, k] = id[p, q] with k contiguous: lets the per-tile diag
        # stack build run at DVE 2x (all operands' last dims are stride-1)
        idK2 = singles.tile([P, P, K], F16)
        nc.vector.tensor_copy(idK2, id16.unsqueeze(2).broadcast_to([P, P, K]))

        # Wsc[d, e] = a_w[d] * W[d, e]
        Wsc0 = singles.tile([P, D], F32)
        Wsc1 = singles.tile([P, D], F32)
        nc.vector.tensor_scalar_mul(Wsc0, W0, awT[:, 0:1])
        nc.vector.tensor_scalar_mul(Wsc1, W1, awT[:, 1:2])
        ones_col = singles.tile([P, 1], F32)
        ones_row = singles.tile([1, P], F32)
        nc.vector.memset(ones_col, 1.0)
        nc.vector.memset(ones_row, 1.0)
        # u[e] = sum_d Wsc[d, e]  (partition reduction via PE)
        u_ps = psums.tile([1, D], F32)
        nc.tensor.matmul(u_ps, ones_col, Wsc0, start=True, stop=False)
        nc.tensor.matmul(u_ps, ones_col, Wsc1, start=False, stop=True)
        u_row = singles.tile([1, D], F32)
        nc.scalar.copy(u_row, u_ps)

        # c = 2*(b . a_w) + a_b
        baw_scr = small.tile([1, D], F32, tag="baw_scr")
        baw = small.tile([1, 1], F32, tag="baw")
        nc.vector.tensor_mul(baw_scr, b_row, aw_row)
        nc.vector.reduce_sum(baw, baw_scr, AX.X)
        c_s = singles.tile([1, 1], F32)
        nc.scalar.activation(c_s, baw, AF.Identity, bias=ab_t, scale=2.0)

        # broadcast u, c across all 128 partitions via PE outer product
        ub_ps = psums.tile([P, D], F32)
        nc.tensor.matmul(ub_ps, ones_row, u_row, start=True, stop=True)
        u_b = singles.tile([P, D], F32)
        nc.scalar.copy(u_b, ub_ps)
        cb_ps = psums.tile([P, 1], F32)
        nc.tensor.matmul(cb_ps, ones_row, c_s, start=True, stop=True)
        c_b = singles.tile([P, 1], F32)
        nc.scalar.copy(c_b, cb_ps)
        u_h = singles.tile([P, D], F16)
        nc.vector.tensor_copy(u_h, u_b)

        # scratch for the target dot-products' full-size out
        scr_v = singles.tile([P, D], F32)

        # The middle-tile cast-DMAs are STRICTLY SERIALIZED: concurrently
        # issued software-DGE DMAs complete round-robin (a burst of Q makes
        # tile 0 arrive after Q full transfers), while one cast-DMA alone
        # already saturates all 16 DMA engines. A tiny gp copy that reads
        # tile t's buffer before issuing tile t+1's DMA forces in-order
        # arrival (~11 us/tile < the 12.5 us/tile DVE pace).
        mh_pre = {}
        pace = singles.tile([P, 1], F16)
        last_mh = [None]

        def dma_mh(t):
            if last_mh[0] is not None:
                nc.gpsimd.tensor_copy(pace, last_mh[0][:, 0, 0:1])
            mh = mids.tile([P, K, D], F16, tag="mid")
            # fp32 -> fp16 cast happens inside the DMA (gpsimd software DGE)
            nc.gpsimd.dma_start(mh, mid[t * P : (t + 1) * P, :, :])
            last_mh[0] = mh
            return mh

        mh_pre[0] = dma_mh(0)

        # ---- stcc[:, t] = target[t*128:(t+1)*128] . u + c ----
        stc = singles.tile([P, nt], F32)
        for t in range(nt):
            tg = small.tile([P, D], F32, tag="tg")
            nc.sync.dma_start(tg, tgt[t * P : (t + 1) * P, :])
            nc.vector.scalar_tensor_tensor(
                out=scr_v, in0=tg, scalar=1.0, in1=u_b,
                op0=ALU.mult, op1=ALU.mult, accum_out=stc[:, t : t + 1],
            )
        stcc = singles.tile([P, nt], F32)
        nc.vector.tensor_scalar_add(stcc, stc, c_b)

        # ---- main loop over node tiles (software-pipelined 2 deep) ----
        # Engine queues execute in emission order. Tile t's aggregation
        # (diag build + matmuls) is deferred into iteration t+1, emitted
        # BETWEEN tile t+1's first big DVE ops, so the DVE queue never
        # stalls on the ACT softmax round-trip; the PSUM evacuation runs a
        # further tile later.
        def scores_head(t):
            """DMA tile t and start its score chain: m2 = mh*u, first add."""
            mh = mh_pre.pop(t) if t in mh_pre else dma_mh(t)
            m2 = tree.tile([P, K, D], F16, tag="m2")
            nc.vector.tensor_mul(
                m2, mh, u_h.unsqueeze(1).broadcast_to([P, K, D])
            )
            a1 = tree.tile([P, K, 128], F16, tag="a1")
            nc.vector.tensor_add(a1, m2[:, :, 0:128], m2[:, :, 128:256])
            return mh, a1

        def scores_tail(t, a1):
            """Finish tile t's scores and softmax (ACT)."""
            a2 = tree.tile([P, K, 64], F16, tag="a2")
            nc.vector.tensor_add(a2, a1[:, :, 0:64], a1[:, :, 64:128])
            a3 = tree.tile([P, K, 32], F16, tag="a3")
            nc.vector.tensor_add(a3, a2[:, :, 0:32], a2[:, :, 32:64])
            s = small.tile([P, K], F32, tag="s")
            nc.vector.reduce_sum(s, a3, AX.X)
            # s2 = leaky_relu(s + (t.u + c)) in ONE ACT op
            s2 = small.tile([P, K], F32, tag="s2")
            nc.scalar.activation(
                s2, s, AF.Lrelu, bias=stcc[:, t : t + 1], scale=1.0,
                alpha=NEG_SLOPE,
            )
            # e = exp(s2) in fp16 (feeds the 2x diag build), den = sum_k e
            # in the instruction's f32 accumulator
            e16 = small.tile([P, K], F16, tag="e16")
            den = small.tile([P, 1], F32, tag="den")
            nc.scalar.activation(e16, s2, AF.Exp, accum_out=den)
            return e16, den

        def aggregate(t, mh, e16, den):
            """Tile t's diag stack (one DVE 2x op) + 32 PE matmuls."""
            rcp = small.tile([P, 1], F32, tag="rcp")
            nc.vector.reciprocal(rcp, den)
            # dgs[p, q, k] = e[p,k] * id[p,q], k contiguous so every
            # operand's last dim is stride-1 (DVE 2x)
            dgs = dgss.tile([P, P, K], F16, tag="dgs")
            nc.vector.tensor_mul(
                dgs, e16.unsqueeze(1).broadcast_to([P, P, K]), idK2
            )
            o_ps = psum.tile([P, D], F32, tag="o_ps")
            for k in range(K):
                nc.tensor.matmul(
                    o_ps, dgs[:, :, k], mh[:, k, :],
                    start=(k == 0), stop=(k == K - 1), skip_group_check=True,
                )
            return o_ps, rcp

        agg = None  # (mh, e16, den, t) awaiting aggregation
        prev = None  # (o_ps, rcp, t) awaiting PSUM evacuation
        for t in range(nt):
            mh, a1 = scores_head(t)
            if agg is not None:
                o_ps, rcp = aggregate(agg[3], agg[0], agg[1], agg[2])
            e16, den = scores_tail(t, a1)
            if agg is not None:
                if prev is not None:
                    _flush(nc, out, outs, prev)
                prev = (o_ps, rcp, agg[3])
            agg = (mh, e16, den, t)

        o_ps, rcp = aggregate(agg[3], agg[0], agg[1], agg[2])
        if prev is not None:
            _flush(nc, out, outs, prev)
        _flush(nc, out, outs, (o_ps, rcp, agg[3]))


def _flush(nc, out, outs, prev):
    o_ps, rcp, t = prev
    o_sb = outs.tile([P, D], F32, tag="o_sb")
    nc.scalar.activation(o_sb, o_ps, AF.Copy, scale=rcp)
    nc.sync.dma_start(out[t * P : (t + 1) * P, :], o_sb)


def build_nc(ns=NS):
    nc = bass.Bass("TRN2", debug=False, num_devices=N_CORES)
    tgt = nc.dram_tensor("target", [ns, D], F32, kind="ExternalInput").ap()
    mid = nc.dram_tensor("middle", [ns, K, D], F32, kind="ExternalInput").ap()
    W = nc.dram_tensor("W", [D, D], F32, kind="ExternalInput").ap()
    b = nc.dram_tensor("b", [D], F32, kind="ExternalInput").ap()
    a_w = nc.dram_tensor("a_w", [1, D], F32, kind="ExternalInput").ap()
    a_b = nc.dram_tensor("a_b", [1], F32, kind="ExternalInput").ap()
    ident = nc.dram_tensor("ident", [P, P], F32, kind="ExternalInput").ap()
    out = nc.dram_tensor("out", [ns, D], F32, kind="ExternalOutput").ap()
    with tile.TileContext(nc) as tc:
        emit_kernel(tc, out, tgt, mid, W, b, a_w, a_b, ident, ns)
    import bass_rust as _br

    # Split multi-wait instructions (walrus allows at most 1 sync wait per
    # instruction; Tile can emit more after multi-DMA dependencies).
    _br.generate_event_semaphores(nc)
    return nc


_NC_CACHE = {}


def _get_nc(ns=NS):
    if ns not in _NC_CACHE:
        _NC_CACHE[ns] = build_nc(ns)
    return _NC_CACHE[ns]


def make_in_maps(target, middle, W, b, a_w, a_b):
    target = np.ascontiguousarray(np.asarray(target, dtype=np.float32))
    middle = np.ascontiguousarray(np.asarray(middle, dtype=np.float32))
    W = np.ascontiguousarray(np.asarray(W, dtype=np.float32))
    b = np.ascontiguousarray(np.asarray(b, dtype=np.float32))
    a_w = np.ascontiguousarray(np.asarray(a_w, dtype=np.float32))
    a_b = np.ascontiguousarray(np.asarray(a_b, dtype=np.float32))
    ident = np.eye(P, dtype=np.float32)
    tgt_shards = np.split(target, N_CORES, axis=0)
    mid_shards = np.split(middle, N_CORES, axis=0)
    return [
        {
            "target": tgt_shards[i],
            "middle": mid_shards[i],
            "W": W,
            "b": b,
            "a_w": a_w,
            "a_b": a_b,
            "ident": ident,
        }
        for i in range(N_CORES)
    ]


def run_sharded(in_maps, **kwargs):
    nc = _get_nc(in_maps[0]["target"].shape[0])
    res = run_bass_kernel_spmd(nc, in_maps, list(range(N_CORES)), **kwargs)
    full = np.concatenate([r["out"] for r in res.results], axis=0)
    return full, res


def kernel(target, middle, W, b, a_w, a_b):
    in_maps = make_in_maps(target, middle, W, b, a_w, a_b)
    full, _ = run_sharded(in_maps)
    return full


# revision 31
# speedup vs baseline: 1.0762x; 1.0762x over previous
"""Trainium2 Bass kernel for AttentionAggregate (GAT-style neighbor aggregation).

Reference computation (per node n, neighbors k=0..K-1):
    pt = target @ W.T + b                      # [N, D]
    pm = middle @ W.T + b                      # [N, K, D]
    score = leaky_relu((pt[:,None,:] + pm) @ a_w.T + a_b)
    coef  = softmax(score, axis=K)
    out   = sum_k coef * middle                # [N, D]

Key algebraic simplification: the W-projection only enters through the dot
with a_w, so with u = a_w @ W (a single D-vector) and c = 2*(a_w.b) + a_b:
    score[n,k] = target[n].u + middle[n,k].u + c
This removes all large matmuls; the kernel is a memory-bound pass over
`middle` (512 MiB) with per-node softmax weighting.

Sharding: data-parallel over nodes. N=16384 nodes split across 8 cores
(2048 nodes each); W/b/a_w/a_b replicated; no cross-core communication.

Per 128-node tile [128, K, D] (node on partition):
  load: `middle` is cast fp32->fp16 IN THE DMA (gpsimd software DGE can
      cast) -- halves SBUF traffic and enables the DVE 2x mode; HBM reads
      are unchanged (still the roofline, ~12 us/tile over 16 DMA engines).
  scores (DVE): m2 = mh * u (u broadcast via a stride-0 AP; fp16 runs at
      DVE 2x), a 3-level tree of fp16 tensor_adds (tensor_tensor runs 2x;
      tensor_reduce has no fast mode), then a [P,K,32]->[P,K] reduce with
      fp32 output (fp32 accumulation keeps score err ~1e-3).
  softmax: leaky_relu(s + target.u + c) in one ACT Lrelu (bias = per-node
      target score, alpha = slope); e = exp(s2) in fp16 with den = sum_k e
      from the same instruction's f32 accumulator; 1/den on DVE. No
      max-subtraction: scores are O(+-8) so exp is safe (also in fp16).
  aggregation (PE, fp16): out = sum_k diag(e[:,k]) @ mh[:,k,:], 32
      accumulating matmuls (1 cycle/row). The diag stack is built half on
      DVE (ONE broadcast-AP tensor_mul for k < KD: dgs[p,k,q] =
      e[p,k]*id[p,q]) and half on ACT (per-k Copy with per-partition
      scale) to balance engine load. PSUM evacuated with the 1/den
      normalization folded in (ACT Copy, scale=rcp), software-pipelined
      one tile late so no engine round-trip blocks the next tile.
"""

from contextlib import ExitStack

import numpy as np

import concourse.bass as bass
import concourse.tile as tile
from concourse import mybir
from concourse.bass_utils import run_bass_kernel_spmd

N_CORES = 8
N, K, D = 16384, 32, 256
NS = N // N_CORES  # nodes per core
P = 128
F32 = mybir.dt.float32
F16 = mybir.dt.float16
ALU = mybir.AluOpType
AF = mybir.ActivationFunctionType
AX = mybir.AxisListType
NEG_SLOPE = 0.01


def emit_kernel(tc, out, tgt, mid, W, b, a_w, a_b, ident, ns):
    nc = tc.nc
    nt = ns // P  # node tiles per core
    with ExitStack() as ctx:
        singles = ctx.enter_context(tc.tile_pool(name="singles", bufs=1))
        mids = ctx.enter_context(tc.tile_pool(name="mids", bufs=5))
        tree = ctx.enter_context(tc.tile_pool(name="tree", bufs=1))
        small = ctx.enter_context(tc.tile_pool(name="small", bufs=3))
        dgss = ctx.enter_context(tc.tile_pool(name="dgss", bufs=2))
        psum = ctx.enter_context(tc.tile_pool(name="psum", bufs=3, space="PSUM"))
        psums = ctx.enter_context(tc.tile_pool(name="psums", bufs=1, space="PSUM"))
        outs = ctx.enter_context(tc.tile_pool(name="outs", bufs=3))

        # ---- setup: u = a_w @ W, c = 2*(a_w.b) + a_b ----
        W0 = singles.tile([P, D], F32)
        W1 = singles.tile([P, D], F32)
        nc.sync.dma_start(W0, W[0:P, :])
        nc.sync.dma_start(W1, W[P : 2 * P, :])
        # a_w transposed onto partitions: awT[p, g] = a_w[0, g*128 + p]
        awT = singles.tile([P, 2], F32)
        nc.sync.dma_start(awT, a_w.rearrange("o (g p) -> p (g o)", g=2))
        b_row = singles.tile([1, D], F32)
        nc.sync.dma_start(b_row, b.unsqueeze(0))
        aw_row = singles.tile([1, D], F32)
        nc.sync.dma_start(aw_row, a_w)
        ab_t = singles.tile([1, 1], F32)
        nc.sync.dma_start(ab_t, a_b.unsqueeze(0))
        id_t = singles.tile([P, P], F32)
        nc.sync.dma_start(id_t, ident)
        id16 = singles.tile([P, P], F16)
        nc.vector.tensor_copy(id16, id_t)
        # idK2[p, q, k] = id[p, q] with k contiguous: lets the per-tile diag
        # stack build run at DVE 2x (all operands' last dims are stride-1)
        idK2 = singles.tile([P, P, K], F16)
        nc.vector.tensor_copy(idK2, id16.unsqueeze(2).broadcast_to([P, P, K]))

        # Wsc[d, e] = a_w[d] * W[d, e]
        Wsc0 = singles.tile([P, D], F32)
        Wsc1 = singles.tile([P, D], F32)
        nc.vector.tensor_scalar_mul(Wsc0, W0, awT[:, 0:1])
        nc.vector.tensor_scalar_mul(Wsc1, W1, awT[:, 1:2])
        ones_col = singles.tile([P, 1], F32)
        ones_row = singles.tile([1, P], F32)
        nc.vector.memset(ones_col, 1.0)
        nc.vector.memset(ones_row, 1.0)
        # u[e] = sum_d Wsc[d, e]  (partition reduction via PE)
        u_ps = psums.tile([1, D], F32)
        nc.tensor.matmul(u_ps, ones_col, Wsc0, start=True, stop=False)
        nc.tensor.matmul(u_ps, ones_col, Wsc1, start=False, stop=True)
        u_row = singles.tile([1, D], F32)
        nc.scalar.copy(u_row, u_ps)

        # c = 2*(b . a_w) + a_b
        baw_scr = small.tile([1, D], F32, tag="baw_scr")
        baw = small.tile([1, 1], F32, tag="baw")
        nc.vector.tensor_mul(baw_scr, b_row, aw_row)
        nc.vector.reduce_sum(baw, baw_scr, AX.X)
        c_s = singles.tile([1, 1], F32)
        nc.scalar.activation(c_s, baw, AF.Identity, bias=ab_t, scale=2.0)

        # broadcast u, c across all 128 partitions via PE outer product
        ub_ps = psums.tile([P, D], F32)
        nc.tensor.matmul(ub_ps, ones_row, u_row, start=True, stop=True)
        u_b = singles.tile([P, D], F32)
        nc.scalar.copy(u_b, ub_ps)
        cb_ps = psums.tile([P, 1], F32)
        nc.tensor.matmul(cb_ps, ones_row, c_s, start=True, stop=True)
        c_b = singles.tile([P, 1], F32)
        nc.scalar.copy(c_b, cb_ps)
        u_h = singles.tile([P, D], F16)
        nc.vector.tensor_copy(u_h, u_b)

        # scratch for the target dot-products' full-size out
        scr_v = singles.tile([P, D], F32)

        # The middle-tile cast-DMAs are STRICTLY SERIALIZED: concurrently
        # issued software-DGE DMAs complete round-robin (a burst of Q makes
        # tile 0 arrive after Q full transfers), while one cast-DMA alone
        # already saturates all 16 DMA engines. A tiny gp copy that reads
        # tile t's buffer before issuing tile t+1's DMA forces in-order
        # arrival (~11 us/tile < the 12.5 us/tile DVE pace).
        mh_pre = {}
        pace = singles.tile([P, 1], F16)
        last_mh = [None]

        def dma_mh(t):
            # pace only the first few issues: a startup burst completes
            # round-robin, delaying tile 0's arrival by the whole burst;
            # in steady state concurrency is harmless (engines stay fed)
            if t in (1, 2) and last_mh[0] is not None:
                nc.gpsimd.tensor_copy(pace, last_mh[0][:, 0, 0:1])
            mh = mids.tile([P, K, D], F16, tag="mid")
            # fp32 -> fp16 cast happens inside the DMA (gpsimd software DGE)
            nc.gpsimd.dma_start(mh, mid[t * P : (t + 1) * P, :, :])
            last_mh[0] = mh
            return mh

        mh_pre[0] = dma_mh(0)

        # ---- stcc[:, t] = target[t*128:(t+1)*128] . u + c ----
        stc = singles.tile([P, nt], F32)
        for t in range(nt):
            tg = small.tile([P, D], F32, tag="tg")
            nc.sync.dma_start(tg, tgt[t * P : (t + 1) * P, :])
            nc.vector.scalar_tensor_tensor(
                out=scr_v, in0=tg, scalar=1.0, in1=u_b,
                op0=ALU.mult, op1=ALU.mult, accum_out=stc[:, t : t + 1],
            )
        stcc = singles.tile([P, nt], F32)
        nc.vector.tensor_scalar_add(stcc, stc, c_b)

        # ---- main loop over node tiles (software-pipelined 2 deep) ----
        # Engine queues execute in emission order. Tile t's aggregation
        # (diag build + matmuls) is deferred into iteration t+1, emitted
        # BETWEEN tile t+1's first big DVE ops, so the DVE queue never
        # stalls on the ACT softmax round-trip; the PSUM evacuation runs a
        # further tile later.
        def scores_head(t):
            """DMA tile t and start its score chain: m2 = mh*u, first add."""
            mh = mh_pre.pop(t) if t in mh_pre else dma_mh(t)
            m2 = tree.tile([P, K, D], F16, tag="m2")
            nc.vector.tensor_mul(
                m2, mh, u_h.unsqueeze(1).broadcast_to([P, K, D])
            )
            a1 = tree.tile([P, K, 128], F16, tag="a1")
            nc.vector.tensor_add(a1, m2[:, :, 0:128], m2[:, :, 128:256])
            return mh, a1

        def scores_tail(t, a1):
            """Finish tile t's scores and softmax (ACT)."""
            a2 = tree.tile([P, K, 64], F16, tag="a2")
            nc.vector.tensor_add(a2, a1[:, :, 0:64], a1[:, :, 64:128])
            a3 = tree.tile([P, K, 32], F16, tag="a3")
            nc.vector.tensor_add(a3, a2[:, :, 0:32], a2[:, :, 32:64])
            s = small.tile([P, K], F32, tag="s")
            nc.vector.reduce_sum(s, a3, AX.X)
            # s2 = leaky_relu(s + (t.u + c)) in ONE ACT op
            s2 = small.tile([P, K], F32, tag="s2")
            nc.scalar.activation(
                s2, s, AF.Lrelu, bias=stcc[:, t : t + 1], scale=1.0,
                alpha=NEG_SLOPE,
            )
            # e = exp(s2) in fp16 (feeds the 2x diag build), den = sum_k e
            # in the instruction's f32 accumulator
            e16 = small.tile([P, K], F16, tag="e16")
            den = small.tile([P, 1], F32, tag="den")
            nc.scalar.activation(e16, s2, AF.Exp, accum_out=den)
            return e16, den

        def aggregate(t, mh, e16, den):
            """Tile t's diag stack (one DVE 2x op) + 32 PE matmuls."""
            rcp = small.tile([P, 1], F32, tag="rcp")
            nc.vector.reciprocal(rcp, den)
            # dgs[p, q, k] = e[p,k] * id[p,q], k contiguous so every
            # operand's last dim is stride-1 (DVE 2x)
            dgs = dgss.tile([P, P, K], F16, tag="dgs")
            nc.vector.tensor_mul(
                dgs, e16.unsqueeze(1).broadcast_to([P, P, K]), idK2
            )
            o_ps = psum.tile([P, D], F32, tag="o_ps")
            for k in range(K):
                nc.tensor.matmul(
                    o_ps, dgs[:, :, k], mh[:, k, :],
                    start=(k == 0), stop=(k == K - 1), skip_group_check=True,
                )
            return o_ps, rcp

        agg = None  # (mh, e16, den, t) awaiting aggregation
        prev = None  # (o_ps, rcp, t) awaiting PSUM evacuation
        for t in range(nt):
            mh, a1 = scores_head(t)
            if agg is not None:
                o_ps, rcp = aggregate(agg[3], agg[0], agg[1], agg[2])
            e16, den = scores_tail(t, a1)
            if agg is not None:
                if prev is not None:
                    _flush(nc, out, outs, prev)
                prev = (o_ps, rcp, agg[3])
            agg = (mh, e16, den, t)

        o_ps, rcp = aggregate(agg[3], agg[0], agg[1], agg[2])
        if prev is not None:
            _flush(nc, out, outs, prev)
        _flush(nc, out, outs, (o_ps, rcp, agg[3]))


def _flush(nc, out, outs, prev):
    o_ps, rcp, t = prev
    o_sb = outs.tile([P, D], F32, tag="o_sb")
    nc.scalar.activation(o_sb, o_ps, AF.Copy, scale=rcp)
    nc.sync.dma_start(out[t * P : (t + 1) * P, :], o_sb)


def build_nc(ns=NS):
    nc = bass.Bass("TRN2", debug=False, num_devices=N_CORES)
    tgt = nc.dram_tensor("target", [ns, D], F32, kind="ExternalInput").ap()
    mid = nc.dram_tensor("middle", [ns, K, D], F32, kind="ExternalInput").ap()
    W = nc.dram_tensor("W", [D, D], F32, kind="ExternalInput").ap()
    b = nc.dram_tensor("b", [D], F32, kind="ExternalInput").ap()
    a_w = nc.dram_tensor("a_w", [1, D], F32, kind="ExternalInput").ap()
    a_b = nc.dram_tensor("a_b", [1], F32, kind="ExternalInput").ap()
    ident = nc.dram_tensor("ident", [P, P], F32, kind="ExternalInput").ap()
    out = nc.dram_tensor("out", [ns, D], F32, kind="ExternalOutput").ap()
    with tile.TileContext(nc) as tc:
        emit_kernel(tc, out, tgt, mid, W, b, a_w, a_b, ident, ns)
    import bass_rust as _br

    # Split multi-wait instructions (walrus allows at most 1 sync wait per
    # instruction; Tile can emit more after multi-DMA dependencies).
    _br.generate_event_semaphores(nc)
    return nc


_NC_CACHE = {}


def _get_nc(ns=NS):
    if ns not in _NC_CACHE:
        _NC_CACHE[ns] = build_nc(ns)
    return _NC_CACHE[ns]


def make_in_maps(target, middle, W, b, a_w, a_b):
    target = np.ascontiguousarray(np.asarray(target, dtype=np.float32))
    middle = np.ascontiguousarray(np.asarray(middle, dtype=np.float32))
    W = np.ascontiguousarray(np.asarray(W, dtype=np.float32))
    b = np.ascontiguousarray(np.asarray(b, dtype=np.float32))
    a_w = np.ascontiguousarray(np.asarray(a_w, dtype=np.float32))
    a_b = np.ascontiguousarray(np.asarray(a_b, dtype=np.float32))
    ident = np.eye(P, dtype=np.float32)
    tgt_shards = np.split(target, N_CORES, axis=0)
    mid_shards = np.split(middle, N_CORES, axis=0)
    return [
        {
            "target": tgt_shards[i],
            "middle": mid_shards[i],
            "W": W,
            "b": b,
            "a_w": a_w,
            "a_b": a_b,
            "ident": ident,
        }
        for i in range(N_CORES)
    ]


def run_sharded(in_maps, **kwargs):
    nc = _get_nc(in_maps[0]["target"].shape[0])
    res = run_bass_kernel_spmd(nc, in_maps, list(range(N_CORES)), **kwargs)
    full = np.concatenate([r["out"] for r in res.results], axis=0)
    return full, res


def kernel(target, middle, W, b, a_w, a_b):
    in_maps = make_in_maps(target, middle, W, b, a_w, a_b)
    full, _ = run_sharded(in_maps)
    return full


# revision 34
# speedup vs baseline: 1.1350x; 1.0546x over previous
"""Trainium2 Bass kernel for AttentionAggregate (GAT-style neighbor aggregation).

Reference computation (per node n, neighbors k=0..K-1):
    pt = target @ W.T + b                      # [N, D]
    pm = middle @ W.T + b                      # [N, K, D]
    score = leaky_relu((pt[:,None,:] + pm) @ a_w.T + a_b)
    coef  = softmax(score, axis=K)
    out   = sum_k coef * middle                # [N, D]

Key algebraic simplification: the W-projection only enters through the dot
with a_w, so with u = a_w @ W (a single D-vector) and c = 2*(a_w.b) + a_b:
    score[n,k] = target[n].u + middle[n,k].u + c
This removes all large matmuls; the kernel is a memory-bound pass over
`middle` (512 MiB) with per-node softmax weighting.

Sharding: data-parallel over nodes. N=16384 nodes split across 8 cores
(2048 nodes each); W/b/a_w/a_b replicated; no cross-core communication.

Per 128-node tile [128, K, D] (node on partition):
  load: `middle` is cast fp32->fp16 IN THE DMA (gpsimd software DGE can
      cast) -- halves SBUF traffic and enables the DVE 2x mode; HBM reads
      are unchanged (still the roofline, ~12 us/tile over 16 DMA engines).
  scores (DVE): m2 = mh * u (u broadcast via a stride-0 AP; fp16 runs at
      DVE 2x), a 3-level tree of fp16 tensor_adds (tensor_tensor runs 2x;
      tensor_reduce has no fast mode), then a [P,K,32]->[P,K] reduce with
      fp32 output (fp32 accumulation keeps score err ~1e-3).
  softmax: leaky_relu(s + target.u + c) in one ACT Lrelu (bias = per-node
      target score, alpha = slope); e = exp(s2) in fp16 with den = sum_k e
      from the same instruction's f32 accumulator; 1/den on DVE. No
      max-subtraction: scores are O(+-8) so exp is safe (also in fp16).
  aggregation (PE, fp16): out = sum_k diag(e[:,k]) @ mh[:,k,:], 32
      accumulating matmuls (1 cycle/row). The diag stack is built half on
      DVE (ONE broadcast-AP tensor_mul for k < KD: dgs[p,k,q] =
      e[p,k]*id[p,q]) and half on ACT (per-k Copy with per-partition
      scale) to balance engine load. PSUM evacuated with the 1/den
      normalization folded in (ACT Copy, scale=rcp), software-pipelined
      one tile late so no engine round-trip blocks the next tile.
"""

from contextlib import ExitStack

import numpy as np

import concourse.bass as bass
import concourse.tile as tile
from concourse import mybir
from concourse.bass_utils import run_bass_kernel_spmd

N_CORES = 8
N, K, D = 16384, 32, 256
NS = N // N_CORES  # nodes per core
P = 128
F32 = mybir.dt.float32
F16 = mybir.dt.float16
ALU = mybir.AluOpType
AF = mybir.ActivationFunctionType
AX = mybir.AxisListType
NEG_SLOPE = 0.01


def emit_kernel(tc, out, tgt, mid, W, b, a_w, a_b, ident, ns):
    nc = tc.nc
    nt = ns // P  # node tiles per core
    with ExitStack() as ctx:
        singles = ctx.enter_context(tc.tile_pool(name="singles", bufs=1))
        mids = ctx.enter_context(tc.tile_pool(name="mids", bufs=2))
        mids16 = ctx.enter_context(tc.tile_pool(name="mids16", bufs=3))
        tree = ctx.enter_context(tc.tile_pool(name="tree", bufs=1))
        small = ctx.enter_context(tc.tile_pool(name="small", bufs=3))
        dgss = ctx.enter_context(tc.tile_pool(name="dgss", bufs=2))
        psum = ctx.enter_context(tc.tile_pool(name="psum", bufs=3, space="PSUM"))
        psums = ctx.enter_context(tc.tile_pool(name="psums", bufs=1, space="PSUM"))
        outs = ctx.enter_context(tc.tile_pool(name="outs", bufs=3))

        # ---- setup: u = a_w @ W, c = 2*(a_w.b) + a_b ----
        W0 = singles.tile([P, D], F32)
        W1 = singles.tile([P, D], F32)
        nc.sync.dma_start(W0, W[0:P, :])
        nc.sync.dma_start(W1, W[P : 2 * P, :])
        # a_w transposed onto partitions: awT[p, g] = a_w[0, g*128 + p]
        awT = singles.tile([P, 2], F32)
        nc.sync.dma_start(awT, a_w.rearrange("o (g p) -> p (g o)", g=2))
        b_row = singles.tile([1, D], F32)
        nc.sync.dma_start(b_row, b.unsqueeze(0))
        aw_row = singles.tile([1, D], F32)
        nc.sync.dma_start(aw_row, a_w)
        ab_t = singles.tile([1, 1], F32)
        nc.sync.dma_start(ab_t, a_b.unsqueeze(0))
        id_t = singles.tile([P, P], F32)
        nc.sync.dma_start(id_t, ident)
        id16 = singles.tile([P, P], F16)
        nc.vector.tensor_copy(id16, id_t)
        # idK2[p, q, k] = id[p, q] with k contiguous: lets the per-tile diag
        # stack build run at DVE 2x (all operands' last dims are stride-1)
        idK2 = singles.tile([P, P, K], F16)
        nc.vector.tensor_copy(idK2, id16.unsqueeze(2).broadcast_to([P, P, K]))

        # Wsc[d, e] = a_w[d] * W[d, e]
        Wsc0 = singles.tile([P, D], F32)
        Wsc1 = singles.tile([P, D], F32)
        nc.vector.tensor_scalar_mul(Wsc0, W0, awT[:, 0:1])
        nc.vector.tensor_scalar_mul(Wsc1, W1, awT[:, 1:2])
        ones_col = singles.tile([P, 1], F32)
        ones_row = singles.tile([1, P], F32)
        nc.vector.memset(ones_col, 1.0)
        nc.vector.memset(ones_row, 1.0)
        # u[e] = sum_d Wsc[d, e]  (partition reduction via PE)
        u_ps = psums.tile([1, D], F32)
        nc.tensor.matmul(u_ps, ones_col, Wsc0, start=True, stop=False)
        nc.tensor.matmul(u_ps, ones_col, Wsc1, start=False, stop=True)
        u_row = singles.tile([1, D], F32)
        nc.scalar.copy(u_row, u_ps)

        # c = 2*(b . a_w) + a_b
        baw_scr = small.tile([1, D], F32, tag="baw_scr")
        baw = small.tile([1, 1], F32, tag="baw")
        nc.vector.tensor_mul(baw_scr, b_row, aw_row)
        nc.vector.reduce_sum(baw, baw_scr, AX.X)
        c_s = singles.tile([1, 1], F32)
        nc.scalar.activation(c_s, baw, AF.Identity, bias=ab_t, scale=2.0)

        # broadcast u, c across all 128 partitions via PE outer product
        ub_ps = psums.tile([P, D], F32)
        nc.tensor.matmul(ub_ps, ones_row, u_row, start=True, stop=True)
        u_b = singles.tile([P, D], F32)
        nc.scalar.copy(u_b, ub_ps)
        cb_ps = psums.tile([P, 1], F32)
        nc.tensor.matmul(cb_ps, ones_row, c_s, start=True, stop=True)
        c_b = singles.tile([P, 1], F32)
        nc.scalar.copy(c_b, cb_ps)
        u_h = singles.tile([P, D], F16)
        nc.vector.tensor_copy(u_h, u_b)

        # scratch for the target dot-products' full-size out
        scr_v = singles.tile([P, D], F32)

        # ---- stcc[:, t] = target[t*128:(t+1)*128] . u + c ----
        stc = singles.tile([P, nt], F32)
        for t in range(nt):
            tg = small.tile([P, D], F32, tag="tg")
            nc.sync.dma_start(tg, tgt[t * P : (t + 1) * P, :])
            nc.vector.scalar_tensor_tensor(
                out=scr_v, in0=tg, scalar=1.0, in1=u_b,
                op0=ALU.mult, op1=ALU.mult, accum_out=stc[:, t : t + 1],
            )
        stcc = singles.tile([P, nt], F32)
        nc.vector.tensor_scalar_add(stcc, stc, c_b)

        # ---- main loop over node tiles (software-pipelined 3 deep) ----
        # Engine queues execute in emission order. Stages per tile:
        #   DMA (hwdge, f32; hardware queues are FIFO so arrivals are
        #       in-order at full concurrent bandwidth)
        #   -> ACT fp32->fp16 convert (ACT has slack; the fp16 copy gives
        #       DVE its 2x mode and the PE 1-cycle/row matmuls)
        #   -> DVE scores -> ACT softmax -> DVE diag stack + PE matmuls
        #       (deferred one iteration so DVE never stalls on ACT)
        #   -> ACT PSUM evacuation (deferred a further iteration).
        def dma_mf(t):
            mf = mids.tile([P, K, D], F32, tag="midf")
            nc.sync.dma_start(mf, mid[t * P : (t + 1) * P, :, :])
            return mf

        def convert(mf):
            mh = mids16.tile([P, K, D], F16, tag="mid16")
            nc.scalar.copy(mh, mf)
            return mh

        def scores_head(t, mh):
            """Start tile t's score chain: m2 = mh*u, first tree add."""
            m2 = tree.tile([P, K, D], F16, tag="m2")
            nc.vector.tensor_mul(
                m2, mh, u_h.unsqueeze(1).broadcast_to([P, K, D])
            )
            a1 = tree.tile([P, K, 128], F16, tag="a1")
            nc.vector.tensor_add(a1, m2[:, :, 0:128], m2[:, :, 128:256])
            return a1

        def scores_tail(t, a1):
            """Finish tile t's scores and softmax (ACT)."""
            a2 = tree.tile([P, K, 64], F16, tag="a2")
            nc.vector.tensor_add(a2, a1[:, :, 0:64], a1[:, :, 64:128])
            a3 = tree.tile([P, K, 32], F16, tag="a3")
            nc.vector.tensor_add(a3, a2[:, :, 0:32], a2[:, :, 32:64])
            s = small.tile([P, K], F32, tag="s")
            nc.vector.reduce_sum(s, a3, AX.X)
            # s2 = leaky_relu(s + (t.u + c)) in ONE ACT op
            s2 = small.tile([P, K], F32, tag="s2")
            nc.scalar.activation(
                s2, s, AF.Lrelu, bias=stcc[:, t : t + 1], scale=1.0,
                alpha=NEG_SLOPE,
            )
            # e = exp(s2) in fp16 (feeds the 2x diag build), den = sum_k e
            # in the instruction's f32 accumulator
            e16 = small.tile([P, K], F16, tag="e16")
            den = small.tile([P, 1], F32, tag="den")
            nc.scalar.activation(e16, s2, AF.Exp, accum_out=den)
            return e16, den

        def aggregate(t, mh, e16, den):
            """Tile t's diag stack (one DVE 2x op) + 32 PE matmuls."""
            rcp = small.tile([P, 1], F32, tag="rcp")
            nc.vector.reciprocal(rcp, den)
            # dgs[p, q, k] = e[p,k] * id[p,q], k contiguous so every
            # operand's last dim is stride-1 (DVE 2x)
            dgs = dgss.tile([P, P, K], F16, tag="dgs")
            nc.vector.tensor_mul(
                dgs, e16.unsqueeze(1).broadcast_to([P, P, K]), idK2
            )
            o_ps = psum.tile([P, D], F32, tag="o_ps")
            for k in range(K):
                nc.tensor.matmul(
                    o_ps, dgs[:, :, k], mh[:, k, :],
                    start=(k == 0), stop=(k == K - 1), skip_group_check=True,
                )
            return o_ps, rcp

        # prologue: DMA tiles 0-1, convert tile 0
        mf_next = dma_mf(1) if nt > 1 else None
        mh_cur = convert(dma_mf(0))

        agg = None  # (mh, e16, den, t) awaiting aggregation
        prev = None  # (o_ps, rcp, t) awaiting PSUM evacuation
        for t in range(nt):
            if t + 2 < nt:
                mf_next2 = dma_mf(t + 2)
            else:
                mf_next2 = None
            # convert tile t+1 at the head of ACT's queue for this
            # iteration (its DMA completed during iteration t-1)
            mh_next = convert(mf_next) if mf_next is not None else None
            a1 = scores_head(t, mh_cur)
            if agg is not None:
                o_ps, rcp = aggregate(agg[3], agg[0], agg[1], agg[2])
            e16, den = scores_tail(t, a1)
            if agg is not None:
                if prev is not None:
                    _flush(nc, out, outs, prev)
                prev = (o_ps, rcp, agg[3])
            agg = (mh_cur, e16, den, t)
            mh_cur, mf_next = mh_next, mf_next2

        o_ps, rcp = aggregate(agg[3], agg[0], agg[1], agg[2])
        if prev is not None:
            _flush(nc, out, outs, prev)
        _flush(nc, out, outs, (o_ps, rcp, agg[3]))


def _flush(nc, out, outs, prev):
    o_ps, rcp, t = prev
    o_sb = outs.tile([P, D], F32, tag="o_sb")
    nc.scalar.activation(o_sb, o_ps, AF.Copy, scale=rcp)
    nc.sync.dma_start(out[t * P : (t + 1) * P, :], o_sb)


def build_nc(ns=NS):
    nc = bass.Bass("TRN2", debug=False, num_devices=N_CORES)
    tgt = nc.dram_tensor("target", [ns, D], F32, kind="ExternalInput").ap()
    mid = nc.dram_tensor("middle", [ns, K, D], F32, kind="ExternalInput").ap()
    W = nc.dram_tensor("W", [D, D], F32, kind="ExternalInput").ap()
    b = nc.dram_tensor("b", [D], F32, kind="ExternalInput").ap()
    a_w = nc.dram_tensor("a_w", [1, D], F32, kind="ExternalInput").ap()
    a_b = nc.dram_tensor("a_b", [1], F32, kind="ExternalInput").ap()
    ident = nc.dram_tensor("ident", [P, P], F32, kind="ExternalInput").ap()
    out = nc.dram_tensor("out", [ns, D], F32, kind="ExternalOutput").ap()
    with tile.TileContext(nc) as tc:
        emit_kernel(tc, out, tgt, mid, W, b, a_w, a_b, ident, ns)
    import bass_rust as _br

    # Split multi-wait instructions (walrus allows at most 1 sync wait per
    # instruction; Tile can emit more after multi-DMA dependencies).
    _br.generate_event_semaphores(nc)
    return nc


_NC_CACHE = {}


def _get_nc(ns=NS):
    if ns not in _NC_CACHE:
        _NC_CACHE[ns] = build_nc(ns)
    return _NC_CACHE[ns]


def make_in_maps(target, middle, W, b, a_w, a_b):
    target = np.ascontiguousarray(np.asarray(target, dtype=np.float32))
    middle = np.ascontiguousarray(np.asarray(middle, dtype=np.float32))
    W = np.ascontiguousarray(np.asarray(W, dtype=np.float32))
    b = np.ascontiguousarray(np.asarray(b, dtype=np.float32))
    a_w = np.ascontiguousarray(np.asarray(a_w, dtype=np.float32))
    a_b = np.ascontiguousarray(np.asarray(a_b, dtype=np.float32))
    ident = np.eye(P, dtype=np.float32)
    tgt_shards = np.split(target, N_CORES, axis=0)
    mid_shards = np.split(middle, N_CORES, axis=0)
    return [
        {
            "target": tgt_shards[i],
            "middle": mid_shards[i],
            "W": W,
            "b": b,
            "a_w": a_w,
            "a_b": a_b,
            "ident": ident,
        }
        for i in range(N_CORES)
    ]


def run_sharded(in_maps, **kwargs):
    nc = _get_nc(in_maps[0]["target"].shape[0])
    res = run_bass_kernel_spmd(nc, in_maps, list(range(N_CORES)), **kwargs)
    full = np.concatenate([r["out"] for r in res.results], axis=0)
    return full, res


def kernel(target, middle, W, b, a_w, a_b):
    in_maps = make_in_maps(target, middle, W, b, a_w, a_b)
    full, _ = run_sharded(in_maps)
    return full
